# revision 1
# baseline (speedup 1.0000x reference)
"""DetectionLoss Bass kernel for Trainium2, data-parallel over 8 NeuronCores.

Strategy (per core, 8 images as 4 image-pairs):
  - layout B: [128 partitions = 2 images x 64 targets, n(preds) free]
  - overlap_x(n,m) = min(relu(x2_n - x1g_m), wg_m) - relu(x1_n - x1g_m)
    relus computed by ScalarE activation (bias = -x1g per partition) while
    evacuating a PE ones-broadcast of the pred-coordinate rows from PSUM.
  - iou > 0.3  <=>  r = inter / ((3/13)(a1+a2)) > 1   (divide-free threshold;
    r via DVE reciprocal_approx_fast, argmax_m iou == argmax_m r exactly)
  - argmax over targets: gpsimd partition_all_reduce(max) per image half,
    one-hot mask = (r == best), gather = PE matmul  coords = tgt^T @ mask.
  - focal BCE + CIoU finalization in n-partitioned layout, batched over all
    8 images; per-image scalar accumulators reduced via a ones-matmul.
Host combines the 8x8 per-image (focal_sum, masked_ciou_sum, n_pos) triples.
"""

import numpy as np

import concourse.bass as bass
import concourse.bass_isa as bass_isa
from concourse.bacc import Bacc
import concourse.mybir as mybir
from concourse.tile import TileContext

ALU = mybir.AluOpType
ACT = mybir.ActivationFunctionType
F32 = mybir.dt.float32
BF16 = mybir.dt.bfloat16

# problem constants (hardcoded per harness contract)
B_FULL = 64
N = 8400
M = 64
NCORES = 8
BC = B_FULL // NCORES          # images per core
P = 128
C = 66                          # free cols per partition in n-part layout
NPAD = P * C                    # 8448
NC = 1024                       # n-chunk (two PSUM banks of fp32)
CHUNKS = [(k * NC, min(NC, NPAD - k * NC)) for k in range((NPAD + NC - 1) // NC)]
SC13 = 3.0 / 13.0               # iou>0.3  <=>  inter > (3/13)(a1+a2)
EPS = 1e-7


PAD_ROW = np.array([-100.0, -100.0, 1.0, 1.0, -30.0], np.float32)


def pad_preds(preds):
    """Host-side: pad [b, N, 5] -> [b, NPAD, 5] with far-box/low-logit rows."""
    out = np.empty((preds.shape[0], NPAD, 5), np.float32)
    out[:, :N] = preds
    out[:, N:] = PAD_ROW
    return out


def _pred_load(nc, tc, preds_d, PRED, b, bslot, bc):
    """DMA padded preds[b] -> PRED image-slot (n = p*66 + c mapping)."""
    pv = PRED.rearrange("p (b c f) -> p b c f", b=bc, f=5)[:, bslot]  # [128,66,5]
    src = preds_d[b].rearrange("(p c) f -> p c f", c=C)
    nc.sync.dma_start(out=pv[:, :], in_=src)


def host_consts():
    """Host-built constants: selector matmul weights + per-partition scalars."""
    import ml_dtypes
    # K=20 bf16 selector: rows 0..9 hi streams, 10..19 lo streams; stream s
    # picks rows {2s (img A), 2s+1 (img B)} from both halves.
    sels = np.zeros((20, 5 * P), np.float32)
    for s in range(5):
        for base in (0, 10):
            sels[base + 2 * s, s * P : s * P + 64] = 1.0
            sels[base + 2 * s + 1, s * P + 64 : (s + 1) * P] = 1.0
    sels = sels.astype(ml_dtypes.bfloat16)
    onesneg = np.zeros((P, 2), np.float32)
    onesneg[:, 0] = 1.0
    onesneg[:, 1] = -1.0
    # fp32 2-row selector for the exact best broadcast
    selb = np.zeros((2, P), np.float32)
    selb[0, 0:64] = 1.0
    selb[1, 64:P] = 1.0
    ident = np.eye(P, dtype=np.float32)
    return sels, onesneg, selb, ident


def build_nc(bc=BC, trn_type=None):
    """Build the per-core Bass program. bc = images per core (even)."""
    pairs = bc // 2
    nc = Bacc() if trn_type is None else Bacc(trn_type=trn_type)
    preds_d = nc.declare_dram_parameter("preds", [bc, NPAD, 5], F32, isOutput=False)
    tgts_d = nc.declare_dram_parameter("targets", [bc, M, 4], F32, isOutput=False)
    sels_d = nc.declare_dram_parameter("sels", [20, 5 * P], BF16, isOutput=False)
    ones_d = nc.declare_dram_parameter("onesneg", [P, 2], F32, isOutput=False)
    selb_d = nc.declare_dram_parameter("selb", [2, P], F32, isOutput=False)
    ident_d = nc.declare_dram_parameter("ident", [P, P], F32, isOutput=False)
    out_d = nc.declare_dram_parameter("out", [1, 3 * bc], F32, isOutput=True)

    with TileContext(nc) as tc:
        with (
            tc.tile_pool(name="const", bufs=1) as cpool,
            tc.tile_pool(name="persist", bufs=1) as ppool,
        ):
            # ---- constants (host-supplied) ----
            # SELS[:, s*128:(s+1)*128]: K=10 selector for stream s
            # (row 2s -> partitions 0:64 (img A), row 2s+1 -> partitions 64:128)
            SELS = cpool.tile([20, 5 * P], BF16, name="SELS")
            nc.sync.dma_start(out=SELS[:, :], in_=sels_d[:, :])
            ON = cpool.tile([P, 2], F32, name="ON")
            nc.sync.dma_start(out=ON[:, :], in_=ones_d[:, :])
            ONES = ON[:, 0:1]
            NEG1 = ON[:, 1:2]
            SEL2F = cpool.tile([2, P], F32, name="SEL2F")
            nc.sync.dma_start(out=SEL2F[:, :], in_=selb_d[:, :])
            IDENT = cpool.tile([P, P], F32, name="IDENT")
            nc.sync.dma_start(out=IDENT[:, :], in_=ident_d[:, :])

            # ---- persistent (all images) ----
            PRED = ppool.tile([P, bc * C * 5], F32, name="PRED")
            X1 = ppool.tile([P, bc * C], F32, name="X1")
            X2 = ppool.tile([P, bc * C], F32, name="X2")
            Y1 = ppool.tile([P, bc * C], F32, name="Y1")
            Y2 = ppool.tile([P, bc * C], F32, name="Y2")
            A1S = ppool.tile([P, bc * C], F32, name="A1S")   # (3/13)*w*h
            MT = ppool.tile([P, bc * 4 * C], F32, name="MT")  # matched tgt coords (hi)
            MT2 = ppool.tile([P, bc * 4 * C], F32, name="MT2")  # lo residual part
            BT = ppool.tile([P, bc * C], F32, name="BT")      # best r per pred
            SC = ppool.tile([P, 3 * bc], F32, name="SC")      # accumulator columns

            with (
                tc.tile_pool(name="stage", bufs=1) as spool,
                tc.tile_pool(name="prep", bufs=2) as qpool,
                tc.tile_pool(name="work", bufs=2) as wpool,
                tc.tile_pool(name="psum", bufs=1, space="PSUM") as pspool,
            ):
                # staging rows, shared across pairs:
                #  STGB rows 0..9: hi(x2A,x2B,x1A,x1B,y2A,y2B,y1A,y1B,a1sA,a1sB)
                #       rows 10..19: bf16 lo residuals of the same
                #  STGF rows 0..15: gathered coords hi(8) + lo(8)
                #       rows 16..17: best rows (A, B)
                STGB = spool.tile([20, NPAD], BF16, name="STGB", bufs=2)
                STGF = spool.tile([18, NPAD], F32, name="STGF")

                for pr in range(pairs):
                    bA, bB = 2 * pr, 2 * pr + 1
                    # ================= prep (n-part layout) =================
                    for bslot in (bA, bB):
                        _pred_load(nc, tc, preds_d, PRED, bslot, bslot, bc)
                    pv = PRED.rearrange("p (b c f) -> p b c f", b=bc, f=5)

                    # per-image coord streams
                    for bslot in (bA, bB):
                        cx = pv[:, bslot, :, 0]
                        cy = pv[:, bslot, :, 1]
                        w = pv[:, bslot, :, 2]
                        h = pv[:, bslot, :, 3]
                        sl = slice(bslot * C, (bslot + 1) * C)
                        WH = qpool.tile([P, C], F32, name="WH", tag="wh", bufs=4)
                        HH = qpool.tile([P, C], F32, name="HH", tag="hh", bufs=4)
                        nc.vector.tensor_scalar(WH[:, :], w, 0.5, None, ALU.mult)
                        nc.vector.tensor_scalar(HH[:, :], h, 0.5, None, ALU.mult)
                        nc.vector.tensor_tensor(X1[:, sl], cx, WH[:, :], ALU.subtract)
                        nc.vector.tensor_tensor(X2[:, sl], cx, WH[:, :], ALU.add)
                        nc.vector.tensor_tensor(Y1[:, sl], cy, HH[:, :], ALU.subtract)
                        nc.vector.tensor_tensor(Y2[:, sl], cy, HH[:, :], ALU.add)
                        nc.vector.scalar_tensor_tensor(
                            A1S[:, sl], w, SC13, h, ALU.mult, ALU.mult
                        )

                    # split to bf16 hi/lo, collapse into rows (n = p*66 + c)
                    for r, T in enumerate((X2, X1, Y2, Y1, A1S)):
                        for j, bslot in enumerate((bA, bB)):
                            tv = T[:, bslot * C : (bslot + 1) * C]
                            THI = qpool.tile([P, C], BF16, name="THI", tag="thi", bufs=4)
                            TLO = qpool.tile([P, C], BF16, name="TLO", tag="tlo", bufs=4)
                            nc.vector.tensor_copy(THI[:, :], tv)
                            nc.vector.tensor_tensor(TLO[:, :], tv, THI[:, :],
                                                    ALU.subtract)
                            for rr, TT_ in ((2 * r + j, THI), (10 + 2 * r + j, TLO)):
                                dst = STGB[rr : rr + 1, :].rearrange(
                                    "o (p c) -> o p c", c=C
                                )
                                nc.sync.dma_start(out=dst, in_=TT_[:, :])

                    # ---- targets: per-partition scalars (A on 0:64, B on 64:128)
                    TGT = qpool.tile([P, 4], F32, name="TGT", tag="tgt", bufs=3)
                    nc.sync.dma_start(out=TGT[0:64, :], in_=tgts_d[bA])
                    nc.sync.dma_start(out=TGT[64:P, :], in_=tgts_d[bB])
                    TWH = qpool.tile([P, 1], F32, name="TWH", tag="twh")
                    THH = qpool.tile([P, 1], F32, name="THH", tag="thh")
                    TX1 = qpool.tile([P, 1], F32, name="TX1", tag="tx1")
                    TY1 = qpool.tile([P, 1], F32, name="TY1", tag="ty1")
                    TX2 = qpool.tile([P, 1], F32, name="TX2", tag="tx2")
                    TY2 = qpool.tile([P, 1], F32, name="TY2", tag="ty2")
                    NX1 = qpool.tile([P, 1], F32, name="NX1", tag="nx1")
                    NY1 = qpool.tile([P, 1], F32, name="NY1", tag="ny1")
                    A2S = qpool.tile([P, 1], F32, name="A2S", tag="a2s")
                    wg = TGT[:, 2:3]
                    hg = TGT[:, 3:4]
                    nc.vector.tensor_scalar(TWH[:, :], wg, 0.5, None, ALU.mult)
                    nc.vector.tensor_scalar(THH[:, :], hg, 0.5, None, ALU.mult)
                    nc.vector.tensor_tensor(TX1[:, :], TGT[:, 0:1], TWH[:, :], ALU.subtract)
                    nc.vector.tensor_tensor(TX2[:, :], TGT[:, 0:1], TWH[:, :], ALU.add)
                    nc.vector.tensor_tensor(TY1[:, :], TGT[:, 1:2], THH[:, :], ALU.subtract)
                    nc.vector.tensor_tensor(TY2[:, :], TGT[:, 1:2], THH[:, :], ALU.add)
                    nc.vector.tensor_scalar(NX1[:, :], TX1[:, :], -1.0, None, ALU.mult)
                    nc.vector.tensor_scalar(NY1[:, :], TY1[:, :], -1.0, None, ALU.mult)
                    nc.vector.scalar_tensor_tensor(
                        A2S[:, :], wg, SC13, hg, ALU.mult, ALU.mult
                    )
                    # gather weights: imgA coords cols 0..3, imgB 4..7; bf16
                    # hi (cols 0..7 of GWB) + lo residual (cols 8..15)
                    GW = qpool.tile([P, 8], F32, name="GW", tag="gw", bufs=3)
                    GWB = qpool.tile([P, 16], BF16, name="GWB", tag="gwb", bufs=3)
                    nc.vector.memset(GW[:, :], 0.0)
                    for q, T in enumerate((TX1, TY1, TX2, TY2)):
                        nc.vector.tensor_copy(GW[0:64, q : q + 1], T[0:64, :])
                        nc.vector.tensor_copy(GW[64:P, 4 + q : 5 + q], T[64:P, :])
                    nc.vector.tensor_copy(GWB[:, 0:8], GW[:, :])
                    nc.vector.tensor_tensor(GWB[:, 8:16], GW[:, :], GWB[:, 0:8],
                                            ALU.subtract)

                    # ================= pairwise chunk loop =================
                    for n0, nc_ in CHUNKS:
                        nbl = nc_ // P  # 128-col transpose blocks in this chunk
                        # PE ones-broadcast of stream rows into PSUM singles
                        PX2 = pspool.tile([P, NC], F32, name="PX2", tag="st", bufs=3)
                        PX1 = pspool.tile([P, NC], F32, name="PX1", tag="st", bufs=3)
                        PY2 = pspool.tile([P, NC], F32, name="PY2", tag="st", bufs=3)
                        PY1 = pspool.tile([P, NC], F32, name="PY1", tag="st", bufs=3)
                        PA1 = pspool.tile([P, NC], F32, name="PA1", tag="st", bufs=3)
                        for j0 in range(0, nc_, 512):
                            jn = min(512, nc_ - j0)
                            rhs = STGB[0:20, n0 + j0 : n0 + j0 + jn]
                            for s, PT_ in enumerate((PX2, PX1, PY2, PY1, PA1)):
                                nc.tensor.matmul(
                                    PT_[:, j0 : j0 + jn],
                                    SELS[:, s * P : (s + 1) * P],
                                    rhs, start=True, stop=True,
                                )
                        # ACT: relu with per-partition bias, PSUM -> SBUF
                        AXB = wpool.tile([P, 2 * NC], F32, name="AXB", tag="axb",
                                         bufs=1)
                        AYB = wpool.tile([P, 2 * NC], F32, name="AYB", tag="ayb",
                                         bufs=1)
                        S3 = wpool.tile([P, NC], F32, name="S3", tag="s3")
                        nc.scalar.activation(AXB[:, 0:nc_], PX2[:, 0:nc_],
                                             ACT.Relu, bias=NX1[:, :])
                        nc.scalar.activation(AXB[:, NC : NC + nc_], PX1[:, 0:nc_],
                                             ACT.Relu, bias=NX1[:, :])
                        nc.scalar.activation(AYB[:, 0:nc_], PY2[:, 0:nc_],
                                             ACT.Relu, bias=NY1[:, :])
                        nc.scalar.activation(AYB[:, NC : NC + nc_], PY1[:, 0:nc_],
                                             ACT.Relu, bias=NY1[:, :])
                        nc.scalar.activation(S3[:, 0:nc_], PA1[:, 0:nc_],
                                             ACT.Identity, bias=A2S[:, :])
                        # DVE: overlaps, inter; gpsimd: ratio
                        CX = wpool.tile([P, NC], F32, name="CX", tag="cx")
                        CY = wpool.tile([P, NC], F32, name="CY", tag="cy")
                        CYR = wpool.tile([P, NC], F32, name="CYR", tag="cyr", bufs=1)
                        INTER = wpool.tile([P, NC], F32, name="INTER", tag="it")
                        Q = wpool.tile([P, NC], F32, name="Q", tag="q", bufs=1)
                        RH = wpool.tile([P, NC], F32, name="RH", tag="rh")
                        MASK = wpool.tile([P, NC], BF16, name="MASK", tag="msk")
                        nc.vector.scalar_tensor_tensor(
                            CX[:, 0:nc_], AXB[:, 0:nc_], wg, AXB[:, NC : NC + nc_],
                            ALU.min, ALU.subtract,
                        )
                        nc.vector.scalar_tensor_tensor(
                            CY[:, 0:nc_], AYB[:, 0:nc_], hg, AYB[:, NC : NC + nc_],
                            ALU.min, ALU.subtract,
                        )
                        nc.scalar.activation(CYR[:, 0:nc_], CY[:, 0:nc_], ACT.Relu)
                        nc.vector.scalar_tensor_tensor(
                            INTER[:, 0:nc_], CX[:, 0:nc_], 0.0, CYR[:, 0:nc_],
                            ALU.max, ALU.mult,
                        )
                        nc.vector.reciprocal_approx_fast(Q[:, 0:nc_], S3[:, 0:nc_])
                        nc.vector.tensor_tensor(RH[:, 0:nc_], INTER[:, 0:nc_],
                                                Q[:, 0:nc_], ALU.mult)
                        # gpsimd: per-image-half max over targets (bcast over
                        # partitions).  The PAR ucode masks channels from
                        # absolute partition 0, so both calls run at offset 0:
                        # shift img B's half down via DMA and the result back.
                        # B-half first: its shift-up DMA overlaps the A-half
                        # reduction on the gpsimd engine.
                        BEST = wpool.tile([P, NC], F32, name="BEST", tag="bst")
                        RHB = wpool.tile([P, NC], F32, name="RHB", tag="rhb",
                                         bufs=1)
                        BESTB = wpool.tile([P, NC], F32, name="BESTB", tag="bstb")
                        nc.sync.dma_start(out=RHB[0:64, 0:nc_], in_=RH[64:P, 0:nc_])
                        nc.gpsimd.partition_all_reduce(
                            BESTB[0:64, 0:nc_], RHB[0:64, 0:nc_], channels=64,
                            reduce_op=bass_isa.ReduceOp.max,
                        )
                        nc.sync.dma_start(out=BEST[64:P, 0:nc_],
                                          in_=BESTB[0:64, 0:nc_])
                        nc.gpsimd.partition_all_reduce(
                            BEST[0:64, 0:nc_], RH[0:64, 0:nc_], channels=64,
                            reduce_op=bass_isa.ReduceOp.max,
                        )
                        nc.sync.dma_start(out=STGF[16:17, n0 : n0 + nc_],
                                          in_=BEST[0:1, 0:nc_])
                        nc.sync.dma_start(out=STGF[17:18, n0 : n0 + nc_],
                                          in_=BESTB[0:1, 0:nc_])
                        nc.vector.tensor_tensor(MASK[:, 0:nc_], RH[:, 0:nc_],
                                                BEST[:, 0:nc_], ALU.is_equal)
                        # PE gather: coords(hi,lo) = GWB^T @ mask -> [16, nc_]
                        GC = pspool.tile([16, NC], F32, name="GC", tag="gc", bufs=1)
                        for j0 in range(0, nc_, 512):
                            jn = min(512, nc_ - j0)
                            nc.tensor.matmul(GC[:, j0 : j0 + jn], GWB[:, :],
                                             MASK[:, j0 : j0 + jn],
                                             start=True, stop=True)
                        # PSUM -> SBUF bounce (compute APs must start at partition
                        # 0/32/64/96; DMA is exempt and lands it on rows 0..15)
                        GCB = wpool.tile([16, NC], F32, name="GCB", tag="gcb", bufs=1)
                        nc.scalar.activation(GCB[:, 0:nc_], GC[:, 0:nc_], ACT.Copy)
                        nc.sync.dma_start(out=STGF[0:16, n0 : n0 + nc_],
                                          in_=GCB[:, 0:nc_])

                    # ============== return to n-part layout ==============
                    for j, bslot in enumerate((bA, bB)):
                        for q in range(4):
                            for dT, r0 in ((MT, 0), (MT2, 8)):
                                dst = dT[:, (bslot * 4 + q) * C : (bslot * 4 + q + 1) * C]
                                src = STGF[r0 + 4 * j + q : r0 + 4 * j + q + 1, :]
                                nc.sync.dma_start(
                                    out=dst,
                                    in_=src.rearrange("o (p c) -> o p c", c=C),
                                )
                        dst = BT[:, bslot * C : (bslot + 1) * C]
                        src = STGF[16 + j : 17 + j, :].rearrange("o (p c) -> o p c", c=C)
                        nc.sync.dma_start(out=dst, in_=src)

            with (
                tc.tile_pool(name="fin", bufs=1) as wpool,
                tc.tile_pool(name="fpsum", bufs=1, space="PSUM") as pspool,
            ):
                # ================= batched finalization =================
                nc.vector.tensor_tensor(MT[:, :], MT[:, :], MT2[:, :], ALU.add)
                pv = PRED.rearrange("p (b c f) -> p b c f", b=bc, f=5)
                L = pv[:, :, :, 4]      # logits [128, bc, 66]
                CXp = pv[:, :, :, 0]
                CYp = pv[:, :, :, 1]
                Wp = pv[:, :, :, 2]
                Hp = pv[:, :, :, 3]
                BCC = bc * C
                mtv = MT.rearrange("p (b q c) -> p b q c", b=bc, q=4)
                GX1 = mtv[:, :, 0]
                GY1 = mtv[:, :, 1]
                GX2 = mtv[:, :, 2]
                GY2 = mtv[:, :, 3]

                def ftile(name, tag=None, bufs=None):
                    return wpool.tile([P, BCC], F32, name=name, tag=tag or name,
                                      bufs=bufs or 1)

                MTC = ppool.tile([P, BCC], F32, name="MTC")   # matched 0/1
                nc.vector.tensor_scalar(MTC[:, :], BT[:, :], 1.0, None, ALU.is_gt)
                bview = lambda t: t.rearrange("p (b c) -> p b c", b=bc)

                # ---- focal ----
                AZ = ftile("AZ"); SP = ftile("SP"); U0 = ftile("U0"); ZT = ftile("ZT")
                BCE = ftile("BCE"); PT = ftile("PT"); SQ = ftile("SQ"); FF = ftile("FF")
                nc.scalar.activation(AZ[:, :], L, ACT.Abs)
                # softplus(-|z|) = ln(1 + exp(-|z|))  (Softplus not in CoreSim)
                nc.scalar.activation(SP[:, :], AZ[:, :], ACT.Exp, scale=-1.0)
                nc.scalar.activation(SP[:, :], SP[:, :], ACT.Ln, bias=1.0)
                nc.vector.scalar_tensor_tensor(U0[:, :], L, 0.0, SP[:, :], ALU.max, ALU.add)
                nc.vector.tensor_tensor(ZT[:, :], L, MTC[:, :], ALU.mult)
                nc.vector.tensor_tensor(BCE[:, :], U0[:, :], ZT[:, :], ALU.subtract)
                nc.scalar.activation(PT[:, :], BCE[:, :], ACT.Exp, scale=-1.0)
                nc.scalar.activation(SQ[:, :], PT[:, :], ACT.Square, bias=NEG1[:, :])
                nc.vector.scalar_tensor_tensor(FF[:, :], SQ[:, :], 0.25, BCE[:, :],
                                               ALU.mult, ALU.mult)
                nc.vector.tensor_reduce(SC[:, 0:bc], bview(FF), mybir.AxisListType.X,
                                        ALU.add)

                # ---- CIoU ----
                T1 = ftile("T1"); T2 = ftile("T2"); T3 = ftile("T3"); T4 = ftile("T4")
                IW = ftile("IW"); IH = ftile("IH"); IN2 = ftile("IN2"); AG = ftile("AG")
                UN = ftile("UN"); QU = ftile("QU"); IOU = ftile("IOU")
                DX = ftile("DX"); DY = ftile("DY"); DG = ftile("DG"); QD = ftile("QD")
                DD = ftile("DD"); DIOU = ftile("DIOU")
                WGE = ftile("WGE"); HGE = ftile("HGE"); QH = ftile("QH"); RG = ftile("RG")
                ATG = ftile("ATG"); ATP = ftile("ATP"); VV = ftile("VV"); DEN = ftile("DEN")
                QA = ftile("QA"); AL = ftile("AL"); AV = ftile("AV"); CIO = ftile("CIO")
                MC = ftile("MC"); A1R = ftile("A1R")

                # intersection with matched boxes
                nc.vector.tensor_tensor(T1[:, :], X1[:, :], GX1, ALU.max)
                nc.vector.tensor_tensor(T2[:, :], X2[:, :], GX2, ALU.min)
                nc.vector.tensor_tensor(IW[:, :], T2[:, :], T1[:, :], ALU.subtract)
                nc.vector.tensor_tensor(T3[:, :], Y1[:, :], GY1, ALU.max)
                nc.vector.tensor_tensor(T4[:, :], Y2[:, :], GY2, ALU.min)
                nc.vector.tensor_tensor(IH[:, :], T4[:, :], T3[:, :], ALU.subtract)
                nc.vector.tensor_scalar(IH[:, :], IH[:, :], 0.0, None, ALU.max)
                nc.vector.scalar_tensor_tensor(IN2[:, :], IW[:, :], 0.0, IH[:, :],
                                               ALU.max, ALU.mult)
                # union = a1 + ag - inter   (A1S = (3/13) a1)
                nc.vector.tensor_tensor(WGE[:, :], GX2, GX1, ALU.subtract)
                nc.vector.tensor_tensor(HGE[:, :], GY2, GY1, ALU.subtract)
                nc.vector.tensor_tensor(AG[:, :], WGE[:, :], HGE[:, :], ALU.mult)
                nc.vector.tensor_scalar(A1R[:, :], A1S[:, :], 13.0 / 3.0, None, ALU.mult)
                nc.vector.tensor_tensor(UN[:, :], A1R[:, :], AG[:, :], ALU.add)
                nc.vector.scalar_tensor_tensor(UN[:, :], UN[:, :], EPS, IN2[:, :],
                                               ALU.add, ALU.subtract)
                nc.vector.reciprocal_approx_fast(QU[:, :], UN[:, :])
                nc.vector.tensor_tensor(IOU[:, :], IN2[:, :], QU[:, :], ALU.mult)
                # enclosing diag
                nc.vector.tensor_tensor(T1[:, :], X1[:, :], GX1, ALU.min)
                nc.vector.tensor_tensor(T2[:, :], X2[:, :], GX2, ALU.max)
                nc.vector.tensor_tensor(DX[:, :], T2[:, :], T1[:, :], ALU.subtract)
                nc.vector.tensor_tensor(T3[:, :], Y1[:, :], GY1, ALU.min)
                nc.vector.tensor_tensor(T4[:, :], Y2[:, :], GY2, ALU.max)
                nc.vector.tensor_tensor(DY[:, :], T4[:, :], T3[:, :], ALU.subtract)
                nc.scalar.activation(T1[:, :], DX[:, :], ACT.Square)
                nc.scalar.activation(T2[:, :], DY[:, :], ACT.Square)
                nc.vector.scalar_tensor_tensor(DG[:, :], T1[:, :], EPS, T2[:, :],
                                               ALU.add, ALU.add)
                nc.vector.reciprocal_approx_fast(QD[:, :], DG[:, :])
                # center distance
                nc.vector.tensor_tensor(T3[:, :], GX1, GX2, ALU.add)
                nc.vector.scalar_tensor_tensor(T3[:, :], T3[:, :], 0.5, CXp,
                                               ALU.mult, ALU.subtract)
                nc.vector.tensor_tensor(T4[:, :], GY1, GY2, ALU.add)
                nc.vector.scalar_tensor_tensor(T4[:, :], T4[:, :], 0.5, CYp,
                                               ALU.mult, ALU.subtract)
                nc.scalar.activation(T3[:, :], T3[:, :], ACT.Square)
                nc.scalar.activation(T4[:, :], T4[:, :], ACT.Square)
                nc.vector.tensor_tensor(DD[:, :], T3[:, :], T4[:, :], ALU.add)
                nc.vector.tensor_tensor(DD[:, :], DD[:, :], QD[:, :], ALU.mult)
                # diou - 1 = dist/diag - iou
                nc.vector.scalar_tensor_tensor(DIOU[:, :], IOU[:, :], -1.0, DD[:, :],
                                               ALU.mult, ALU.add)
                # aspect term.  ScalarE Arctan domain is [-pi/2, pi/2], so use
                # arctan(x) = a + 1[x>1]*(pi/2 - 2a),  a = arctan(min(x, 1/x)).
                def atan_pos(dst, x, ta, tb):
                    nc.vector.tensor_scalar(ta[:, :], x[:, :], 1e-20, None, ALU.max)
                    nc.vector.reciprocal_approx_fast(tb[:, :], ta[:, :])
                    nc.vector.tensor_tensor(tb[:, :], ta[:, :], tb[:, :], ALU.min)
                    nc.scalar.activation(dst[:, :], tb[:, :], ACT.Arctan)
                    nc.vector.tensor_scalar(ta[:, :], ta[:, :], 1.0, None, ALU.is_gt)
                    nc.vector.tensor_scalar(tb[:, :], dst[:, :], -2.0, float(np.pi / 2),
                                            ALU.mult, ALU.add)
                    nc.vector.tensor_tensor(ta[:, :], ta[:, :], tb[:, :], ALU.mult)
                    nc.vector.tensor_tensor(dst[:, :], dst[:, :], ta[:, :], ALU.add)

                nc.vector.tensor_scalar(T1[:, :], HGE[:, :], 1e-12, None, ALU.max)
                nc.vector.reciprocal_approx_fast(QH[:, :], T1[:, :])
                nc.vector.tensor_tensor(RG[:, :], WGE[:, :], QH[:, :], ALU.mult)
                atan_pos(ATG, RG, T1, T2)
                nc.vector.tensor_scalar(T2[:, :], Hp, 1e-12, None, ALU.max)
                nc.vector.reciprocal_approx_fast(QH[:, :], T2[:, :])
                nc.vector.scalar_tensor_tensor(RG[:, :], QH[:, :], 1.0, Wp,
                                               ALU.mult, ALU.mult)
                atan_pos(ATP, RG, T1, T2)
                nc.vector.tensor_tensor(T3[:, :], ATG[:, :], ATP[:, :], ALU.subtract)
                nc.scalar.activation(VV[:, :], T3[:, :], ACT.Square,
                                     scale=2.0 / np.pi)
                # alpha = v / (1 - iou + v + eps)
                nc.vector.tensor_tensor(DEN[:, :], VV[:, :], IOU[:, :], ALU.subtract)
                nc.vector.tensor_scalar(DEN[:, :], DEN[:, :], 1.0 + EPS, None, ALU.add)
                nc.vector.reciprocal_approx_fast(QA[:, :], DEN[:, :])
                nc.vector.tensor_tensor(AL[:, :], VV[:, :], QA[:, :], ALU.mult)
                nc.vector.tensor_tensor(AV[:, :], AL[:, :], VV[:, :], ALU.mult)
                # ciou = 1 + (diou - 1) + alpha*v
                nc.vector.scalar_tensor_tensor(CIO[:, :], DIOU[:, :], 1.0, AV[:, :],
                                               ALU.add, ALU.add)
                nc.vector.tensor_tensor(MC[:, :], CIO[:, :], MTC[:, :], ALU.mult)
                nc.vector.tensor_reduce(SC[:, bc : 2 * bc], bview(MC),
                                        mybir.AxisListType.X, ALU.add)
                nc.vector.tensor_reduce(SC[:, 2 * bc : 3 * bc], bview(MTC),
                                        mybir.AxisListType.X, ALU.add)

                # ---- cross-partition reduce + output ----
                PS = pspool.tile([1, 3 * bc], F32, name="PS", tag="ps")
                nc.tensor.matmul(PS[:, :], ONES[:, :], SC[:, :], start=True, stop=True)
                OUTS = wpool.tile([1, 3 * bc], F32, name="OUTS", tag="outs")
                nc.scalar.activation(OUTS[:, :], PS[:, :], ACT.Copy)
                nc.sync.dma_start(out=out_d[:, :], in_=OUTS[:, :])

    nc.finalize()
    return nc


# ---------------- host side ----------------
_CACHE = {}


def _get_nc():
    if "nc" not in _CACHE:
        _CACHE["nc"] = build_nc()
    return _CACHE["nc"]


def combine(per_img):
    """per_img [B, 3] float64: (focal_sum, masked_ciou_sum, n_pos) -> loss."""
    f = per_img[:, 0] / float(N)
    conf = f.mean()
    npos = per_img[:, 2]
    per_box = per_img[:, 1] / np.maximum(npos, 1.0)
    has = (npos > 0).astype(np.float64)
    nimg = has.sum()
    box = (per_box * has).sum() / max(nimg, 1.0)
    return np.float32(conf + 7.5 * box)


def run(preds, targets, **spmd_kwargs):
    from concourse.bass_utils import run_bass_kernel_spmd

    preds = np.ascontiguousarray(preds, np.float32)
    targets = np.ascontiguousarray(targets, np.float32)
    nc = _get_nc()
    sels, onesneg, selb, ident = host_consts()
    in_maps = [
        {
            "preds": pad_preds(preds[c * BC : (c + 1) * BC]),
            "targets": np.ascontiguousarray(targets[c * BC : (c + 1) * BC]),
            "sels": sels,
            "onesneg": onesneg,
            "selb": selb,
            "ident": ident,
        }
        for c in range(NCORES)
    ]
    res = run_bass_kernel_spmd(nc, in_maps, list(range(NCORES)), **spmd_kwargs)
    rows = []
    for c in range(NCORES):
        o = np.asarray(res.results[c]["out"], np.float64).reshape(3, BC)
        rows.append(o.T)  # [BC, 3]
    per_img = np.concatenate(rows, 0)
    return per_img, res


def kernel(preds, targets):
    per_img, _ = run(preds, targets)
    return combine(per_img)

